# revision 17
# baseline (speedup 1.0000x reference)
"""DiT block kernel for 8 trn2 NeuronCores.

Sharding: core c -> (batch b=c//2, query-token half h=c%2). Each core
computes the full block for its 512 query tokens (K/V for all 1024
tokens of its batch replicated within the pair) -> zero collectives.

Activations feature-major ([D on partitions, tokens on free]). All big
GEMMs run in fp8e4 with DoubleRow perf mode (256-deep contraction per
instruction, 0.5 cyc/row); LN stats in fp16; fp32 PSUM everywhere.
Softmax-invariance drops the k bias exactly; the v bias folds into the
proj bias on the host (softmax rows sum to 1). rsqrt = exp(-0.5*ln(var))
and tanh via exp keep the Act engine on a single ln+exp table until the
final gelu block.
"""
import numpy as np
import ml_dtypes

import concourse.bass as bass
import concourse.tile as tile
import concourse.mybir as mybir
from concourse.bass_utils import run_bass_kernel_spmd
from concourse.vector_clock import ScopedClock
from concourse.alu_op_type import AluOpType

dt = mybir.dt
AF = mybir.ActivationFunctionType
PM = mybir.MatmulPerfMode

P = 128
B, NT, D, H = 4, 1024, 1024, 16
DH = D // H            # 64
DFF = 4 * D            # 4096
KC = D // P            # 8
KP = KC // 2           # 4 chunk pairs (DoubleRow)
LT = NT // 2           # 512 local query tokens
GATE = 0.1
EPS = 1e-5
EXP_SCALE = DH ** -0.5
EXP_BIAS = -1.0


class SplitDrainTileContext(tile.TileContext):
    """Tail drain in this walrus build holds few sync waits; spill the
    rest onto chained SP nops (runs before the sem-clear barrier, so
    semantics are preserved)."""

    MAX_TAIL_WAITS = 1

    def _drain_and_barrier(self, tick_clock, wait_clock):
        drain_inst = self.nc.sync.drain()
        wait_clock.add_sem_waits(
            drain_inst.ins, ScopedClock({None: tick_clock.global_clock})
        )
        si = drain_inst.ins.sync_info
        waits = list(si.on_wait) if si else []
        if len(waits) > self.MAX_TAIL_WAITS:
            drain_inst.ins.sync_info = mybir.SyncInfo(
                on_wait=waits[: self.MAX_TAIL_WAITS],
                on_update=list(si.on_update) if si else [],
            )
            rest = waits[self.MAX_TAIL_WAITS:]
            for i in range(0, len(rest), self.MAX_TAIL_WAITS):
                nop = self.nc.sync.nop()
                nop.ins.sync_info = mybir.SyncInfo(
                    on_wait=rest[i : i + self.MAX_TAIL_WAITS], on_update=[]
                )
        self.nc.all_engine_barrier()
        assert self.sems is not None
        popped = self.nc._tile_sem_poison_stack.pop()
        assert popped is self._sem_poison
        self.nc.clear_and_free_semaphores(list(self.sems.allocated().values()))
        self.nc.all_engine_barrier()


def _legalize_waits(nc, max_waits=1):
    """This walrus build accepts at most one sync wait per instruction.
    Move surplus waits onto same-engine NoOps inserted just before the
    offending instruction (engine FIFO order preserves semantics)."""
    fix = 0
    for bb in nc.main_func.blocks:
        insts = list(bb.instructions)
        out = []
        for inst in insts:
            si = inst.sync_info
            waits = list(si.on_wait) if si else []
            if len(waits) > max_waits:
                keep = waits[-max_waits:]
                for w in waits[:-max_waits]:
                    nop = mybir.InstNoOp(name=f"I-wfix{fix}")
                    fix += 1
                    nop.engine = inst.engine
                    nop.sync_info = mybir.SyncInfo(on_wait=[w], on_update=[])
                    out.append(nop)
                inst.sync_info = mybir.SyncInfo(
                    on_wait=keep, on_update=list(si.on_update) if si else [])
            out.append(inst)
        if len(out) != len(insts):
            bb.instructions = out
    return fix


def _build():
    nc = bass.Bass(target_bir_lowering=False, debug=False,
                   dynamic_dma_scratch_size=2048)
    f32, f16, f8 = dt.float32, dt.float16, dt.float8e4

    xt16 = nc.dram_tensor("xt16", [D, NT], f16, kind="ExternalInput")
    cond8d = nc.dram_tensor("cond8d", [P, KC], f8, kind="ExternalInput")
    qw8d = nc.dram_tensor("qw8d", [P, KP * 2 * D], f8, kind="ExternalInput")
    kw8d = nc.dram_tensor("kw8d", [P, KP * 2 * D], f8, kind="ExternalInput")
    vw8d = nc.dram_tensor("vw8d", [P, KP * 2 * D], f8, kind="ExternalInput")
    pw8d = nc.dram_tensor("pw8d", [DH, KC * 2 * D], f8, kind="ExternalInput")
    f1w8d = nc.dram_tensor("f1w8d", [P, KP * 2 * DFF], f8, kind="ExternalInput")
    f2w8d = nc.dram_tensor("f2w8d", [P, 16 * 2 * D], f8, kind="ExternalInput")
    modw8d = nc.dram_tensor("modw8d", [P, 6 * KP * 2 * D], f8,
                            kind="ExternalInput")
    qbf = nc.dram_tensor("qbf", [P, KC], f32, kind="ExternalInput")
    f1bf = nc.dram_tensor("f1bf", [P, 32], f32, kind="ExternalInput")
    pbrow = nc.dram_tensor("pbrow", [1, D], f16, kind="ExternalInput")
    f2brow = nc.dram_tensor("f2brow", [1, D], f16, kind="ExternalInput")
    modbf = nc.dram_tensor("modbf", [P, 6 * KC], f32, kind="ExternalInput")
    lnf = nc.dram_tensor("lnf", [P, 4 * KC], f32, kind="ExternalInput")
    outt = nc.dram_tensor("outt", [D, LT], f16, kind="ExternalOutput")

    with SplitDrainTileContext(nc) as tc:
        with tc.tile_pool(name="cp", bufs=1) as cp, \
             tc.tile_pool(name="ar", bufs=1) as ar, \
             tc.tile_pool(name="rot", bufs=4) as rot, \
             tc.tile_pool(name="psA", bufs=2, space="PSUM") as psA, \
             tc.tile_pool(name="psB", bufs=3, space="PSUM") as psB:

            def pp():    # [P, 512] f32 psum
                return psA.tile([P, 512], f32, tag="pp", name="pp")

            def pbig():  # [P, 1024] f32 psum
                return psB.tile([P, 1024], f32, tag="big", name="big")

            def r32():   # [P, 1024] f32 sbuf scratch
                return rot.tile([P, NT], f32, tag="R32", bufs=4, name="r32")

            def r16():   # [P, 1024] f16 sbuf scratch
                return rot.tile([P, NT], f16, tag="R16", bufs=4, name="r16")

            def w8():    # [P, 8192] fp8 streaming weight slot
                return ar.tile([P, KP, 2, D], f8, tag="W8", bufs=4, name="w8")

            # ---- constants ----
            ones16 = cp.tile([P, P], f16, tag="ones16")
            nc.vector.memset(ones16[:], 1.0)
            ones8 = cp.tile([P, 2, P], f8, tag="ones8")
            nc.vector.memset(ones8[:], 1.0)
            onesrow = cp.tile([1, LT], f16, tag="onesrow")
            nc.vector.memset(onesrow[:], 1.0)
            expb = cp.tile([P, 1], f32, tag="expb")
            nc.vector.memset(expb[:], EXP_BIAS)
            epsb = cp.tile([P, 1], f32, tag="epsb")
            nc.vector.memset(epsb[:], EPS)
            zerob = cp.tile([P, 1], f32, tag="zerob")
            nc.vector.memset(zerob[:], 0.0)

            # ---- small resident inputs ----
            cond8 = cp.tile([P, KC], f8, tag="cond8")
            nc.sync.dma_start(cond8[:], cond8d[:])
            qbt = cp.tile([P, KC], f32, tag="qbt")
            nc.sync.dma_start(qbt[:], qbf[:])
            f1bt = cp.tile([P, 32], f32, tag="f1bt")
            nc.sync.dma_start(f1bt[:], f1bf[:])
            pbt = cp.tile([1, D], f16, tag="pbt")
            nc.sync.dma_start(pbt[:], pbrow[:])
            f2bt = cp.tile([1, D], f16, tag="f2bt")
            nc.sync.dma_start(f2bt[:], f2brow[:])
            modbt = cp.tile([P, 6, KC], f32, tag="modbt")
            nc.sync.dma_start(modbt[:], modbf.rearrange("p (w c) -> p w c", c=KC))
            lnt = cp.tile([P, 4, KC], f32, tag="lnt")
            nc.sync.dma_start(lnt[:], lnf.rearrange("p (w c) -> p w c", c=KC))

            # ---- big DMAs, in need order ----
            def modw_tile(w):
                t = ar.tile([P, KP, 2, D], f8, tag="MW", bufs=2, name=f"mw{w}")
                nc.sync.dma_start(
                    t[:], modw8d[:, w * KP * 2 * D:(w + 1) * KP * 2 * D]
                    .rearrange("p (k i m) -> p k i m", i=2, m=D))
                return t

            mw0 = modw_tile(0)
            mw1 = modw_tile(1)
            xf = ar.tile([P, KC, NT], f16, tag="XF")
            nc.sync.dma_start(xf[:], xt16.rearrange("(c p) t -> p c t", p=P))
            qw = w8()
            nc.sync.dma_start(qw[:], qw8d.rearrange("p (k i m) -> p k i m",
                                                    i=2, m=D))
            kw = w8()
            nc.sync.dma_start(kw[:], kw8d.rearrange("p (k i m) -> p k i m",
                                                    i=2, m=D))

            # ---- modulation matvecs (PE-cheap) ----
            modv = cp.tile([P, 6, KC], f32, tag="modv")

            def mod_matvec(w, wt):
                # column chains must stay contiguous: a start=True zeroes the
                # whole 2KB PSUM zero-region, so interleaving chains loses
                # earlier columns' partial sums
                pm = pp()
                for mg in range(KC):
                    for j in range(KC):
                        nc.tensor.matmul(pm[:, mg:mg + 1],
                                         wt[:, j // 2, j % 2,
                                            mg * P:(mg + 1) * P],
                                         cond8[:, j:j + 1],
                                         start=(j == 0), stop=(j == KC - 1),
                                         skip_group_check=True)
                nc.vector.tensor_tensor(modv[:, w], pm[:, 0:KC], modbt[:, w],
                                        AluOpType.add)

            vecs = cp.tile([P, 6, KC], f32, tag="vecs")
            negs = cp.tile([P, 2, KC], f32, tag="negs")
            tgp = cp.tile([P, 2, KC], f32, tag="tgp")

            def scale_shift(ln_base, w_gamma, w_beta, col):
                # vecs[col] = ln_g*(1+gamma); vecs[col+1] = ln_b*(1+gamma)+beta
                nc.vector.tensor_scalar_add(tgp[:, col // 3], modv[:, w_gamma],
                                            1.0)
                nc.vector.tensor_tensor(vecs[:, col], tgp[:, col // 3],
                                        lnt[:, ln_base], AluOpType.mult)
                nc.vector.tensor_scalar_mul(negs[:, col // 3], vecs[:, col],
                                            -1.0)
                nc.vector.tensor_tensor(vecs[:, col + 1], tgp[:, col // 3],
                                        lnt[:, ln_base + 1], AluOpType.mult)
                nc.vector.tensor_tensor(vecs[:, col + 1], vecs[:, col + 1],
                                        modv[:, w_beta], AluOpType.add)

            def alpha(w, col):
                # vecs[col] = GATE*tanh(modv[w]) via exp (stays on exp table)
                texp = cp.tile([P, KC], f32, tag=f"texp{col}")
                nc.scalar.activation(texp[:], modv[:, w], AF.Exp, scale=2.0,
                                     bias=zerob[:])
                nc.vector.tensor_scalar_add(texp[:], texp[:], 1.0)
                tr = cp.tile([P, KC], f32, tag=f"trec{col}")
                nc.vector.reciprocal(tr[:], texp[:])
                nc.vector.tensor_scalar(vecs[:, col], tr[:], -2.0 * GATE, GATE,
                                        AluOpType.mult, AluOpType.add)

            mod_matvec(0, mw0)
            mod_matvec(1, mw1)
            scale_shift(0, 0, 1, 0)

            # ---- LN1: stats from f16 xf (PE ones-matmul), 1024 tokens ----
            sq = [None] * KC
            pss = pbig()
            psq = pbig()
            for j in range(KC):
                s16 = r16()
                nc.vector.tensor_tensor(s16[:], xf[:, j], xf[:, j],
                                        AluOpType.mult)
                for nh in range(2):
                    sl = slice(nh * 512, (nh + 1) * 512)
                    nc.tensor.matmul(pss[:, sl], ones16[:], xf[:, j, sl],
                                     start=(j == 0), stop=(j == KC - 1),
                                     skip_group_check=True)
                    nc.tensor.matmul(psq[:, sl], ones16[:], s16[:, sl],
                                     start=(j == 0), stop=(j == KC - 1),
                                     skip_group_check=True)

            def ln_post(pss_, psq_, ntok):
                """-> (arep16, brep16): 1/sigma and mu/sigma, f16 [P, ntok]."""
                mur = r32()
                nc.vector.tensor_scalar_mul(mur[:, 0:ntok], pss_[:, 0:ntok],
                                            1.0 / D)
                msq = r32()
                nc.vector.tensor_scalar_mul(msq[:, 0:ntok], psq_[:, 0:ntok],
                                            1.0 / D)
                mu2 = r32()
                nc.vector.tensor_tensor(mu2[:, 0:ntok], mur[:, 0:ntok],
                                        mur[:, 0:ntok], AluOpType.mult)
                var = r32()
                nc.vector.tensor_tensor(var[:, 0:ntok], msq[:, 0:ntok],
                                        mu2[:, 0:ntok], AluOpType.subtract)
                lnv = r32()
                nc.scalar.activation(lnv[:, 0:ntok], var[:, 0:ntok], AF.Ln,
                                     bias=epsb[:])
                arep = r16()
                nc.scalar.activation(arep[:, 0:ntok], lnv[:, 0:ntok], AF.Exp,
                                     scale=-0.5, bias=zerob[:])
                brep = r16()
                nc.vector.tensor_tensor(brep[:, 0:ntok], mur[:, 0:ntok],
                                        arep[:, 0:ntok], AluOpType.mult)
                return arep, brep

            arep1, brep1 = ln_post(pss, psq, NT)

            # ---- LN1 apply -> y8 (fp8), needs vecs[0,1] ----
            y8 = ar.tile([P, KC, NT], f8, tag="Y8")
            for j in range(KC):
                u = r16()
                nc.vector.scalar_tensor_tensor(u[:], xf[:, j],
                                               vecs[:, 0, j:j + 1],
                                               arep1[:],
                                               AluOpType.mult, AluOpType.mult)
                v = r16()
                nc.vector.scalar_tensor_tensor(v[:], brep1[:],
                                               negs[:, 0, j:j + 1],
                                               u[:],
                                               AluOpType.mult, AluOpType.add)
                nc.vector.tensor_scalar_add(y8[:, j], v[:],
                                            vecs[:, 1, j:j + 1])

            # ---- qkv ----
            vw = w8()
            nc.sync.dma_start(vw[:], vw8d.rearrange("p (k i m) -> p k i m",
                                                    i=2, m=D))
            mw2 = modw_tile(2)

            q8 = ar.tile([P, KC, LT], f8, tag="Q8")
            for mt in range(KC):
                pq = pp()
                for kp in range(KP):
                    nc.tensor.matmul(pq[:], qw[:, kp, :, mt * P:(mt + 1) * P],
                                     y8[:, 2 * kp:2 * kp + 2, 0:LT],
                                     start=(kp == 0), stop=(kp == KP - 1),
                                     perf_mode=PM.DoubleRow)
                nc.vector.tensor_scalar_add(q8[:, mt], pq[:],
                                            qbt[:, mt:mt + 1])

            k8 = ar.tile([P, KC, NT], f8, tag="K8")
            for mt in range(KC):
                pk = pbig()
                for kp in range(KP):
                    for nh in range(2):
                        sl = slice(nh * 512, (nh + 1) * 512)
                        nc.tensor.matmul(pk[:, sl],
                                         kw[:, kp, :, mt * P:(mt + 1) * P],
                                         y8[:, 2 * kp:2 * kp + 2, sl],
                                         start=(kp == 0), stop=(kp == KP - 1),
                                         perf_mode=PM.DoubleRow,
                                         skip_group_check=True)
                nc.vector.tensor_copy(k8[:, mt], pk[:])

            v8 = ar.tile([P, KC, D], f8, tag="V8")
            for tt in range(KC):
                pv = pbig()
                for kp in range(KP):
                    for nh in range(2):
                        sl = slice(nh * 512, (nh + 1) * 512)
                        nc.tensor.matmul(pv[:, sl],
                                         y8[:, 2 * kp:2 * kp + 2,
                                            tt * P:(tt + 1) * P],
                                         vw[:, kp, :, sl],
                                         start=(kp == 0), stop=(kp == KP - 1),
                                         perf_mode=PM.DoubleRow,
                                         skip_group_check=True)
                nc.vector.tensor_copy(v8[:, tt], pv[:])

            mod_matvec(2, mw2)
            alpha(2, 2)
            pw = ar.tile([DH, KC, 2, D], f8, tag="PW")
            nc.sync.dma_start(pw[:], pw8d.rearrange("p (k i m) -> p k i m",
                                                    i=2, m=D))
            mw3 = modw_tile(3)

            # ---- attention: head pair (2g, 2g+1) per feature tile g ----
            # attn64[p, h, t] = attention output feature h*64+p, query t
            attn8 = ar.tile([DH, H, LT], f8, tag="AT")
            for g in range(KC):
                eg = ar.tile([P, KC, NT], f8, tag="EG", bufs=2, name="eg")
                for c in range(KC):
                    psc = pbig()
                    nc.tensor.matmul(psc[:, 0:512],
                                     k8[0:DH, g, c * P:(c + 1) * P],
                                     q8[0:DH, g, :], start=True, stop=True,
                                     skip_group_check=True)
                    nc.tensor.matmul(psc[:, 512:1024],
                                     k8[DH:P, g, c * P:(c + 1) * P],
                                     q8[DH:P, g, :], start=True, stop=True,
                                     skip_group_check=True)
                    nc.scalar.activation(eg[:, c], psc[:], AF.Exp,
                                         scale=EXP_SCALE, bias=expb[:])
                pse = pbig()
                for cp_ in range(KP):
                    for nh in range(2):
                        nc.tensor.matmul(
                            pse[:, nh * 512:(nh + 1) * 512], ones8[:],
                            eg[:, 2 * cp_:2 * cp_ + 2, nh * 512:(nh + 1) * 512],
                            start=(cp_ == 0), stop=(cp_ == KP - 1),
                            perf_mode=PM.DoubleRow, skip_group_check=True)
                recip = r32()
                nc.vector.reciprocal(recip[:], pse[:])
                pav = pbig()  # head 2g in cols 0:512, head 2g+1 in 512:1024
                for nh in range(2):
                    for cp_ in range(KP):
                        nc.tensor.matmul(
                            pav[0:DH, nh * 512:(nh + 1) * 512],
                            v8[:, 2 * cp_:2 * cp_ + 2,
                               (2 * g + nh) * DH:(2 * g + nh + 1) * DH],
                            eg[:, 2 * cp_:2 * cp_ + 2,
                               nh * 512:(nh + 1) * 512],
                            start=(cp_ == 0), stop=(cp_ == KP - 1),
                            perf_mode=PM.DoubleRow, skip_group_check=True)
                for nh in range(2):
                    nc.vector.tensor_tensor(
                        attn8[:, 2 * g + nh], pav[0:DH, nh * 512:(nh + 1) * 512],
                        recip[0:DH, nh * 512:(nh + 1) * 512], AluOpType.mult)

            # ---- proj + gated residual -> x2 (f16) ----
            x2 = ar.tile([P, KC, LT], f16, tag="X2")
            for mt in range(KC):
                pj = pp()
                for c8 in range(KC):
                    nc.tensor.matmul(pj[:], pw[:, c8, :, mt * P:(mt + 1) * P],
                                     attn8[:, 2 * c8:2 * c8 + 2, :],
                                     start=(c8 == 0), stop=False,
                                     perf_mode=PM.DoubleRow)
                nc.tensor.matmul(pj[:], pbt[:, mt * P:(mt + 1) * P],
                                 onesrow[:], start=False, stop=True)
                nc.vector.scalar_tensor_tensor(x2[:, mt], pj[:],
                                               vecs[:, 2, mt:mt + 1],
                                               xf[:, mt, 0:LT],
                                               AluOpType.mult, AluOpType.add)

            mod_matvec(3, mw3)
            mw4 = modw_tile(4)
            mod_matvec(4, mw4)
            scale_shift(2, 3, 4, 3)
            mw5 = modw_tile(5)
            mod_matvec(5, mw5)
            alpha(5, 5)

            # ---- LN2 on x2 (512 tokens) ----
            pss2 = pp()
            psq2 = pp()
            for j in range(KC):
                s16 = r16()
                nc.vector.tensor_tensor(s16[:, 0:LT], x2[:, j], x2[:, j],
                                        AluOpType.mult)
                nc.tensor.matmul(pss2[:], ones16[:], x2[:, j],
                                 start=(j == 0), stop=(j == KC - 1),
                                 skip_group_check=True)
                nc.tensor.matmul(psq2[:], ones16[:], s16[:, 0:LT],
                                 start=(j == 0), stop=(j == KC - 1),
                                 skip_group_check=True)
            arep2, brep2 = ln_post(pss2, psq2, LT)

            f1a = w8()
            nc.sync.dma_start(f1a[:],
                              f1w8d[:, 0:KP * 2 * D]
                              .rearrange("p (k i m) -> p k i m", i=2, m=D))
            f1b = w8()
            nc.sync.dma_start(f1b[:],
                              f1w8d[:, KP * 2 * D:2 * KP * 2 * D]
                              .rearrange("p (k i m) -> p k i m", i=2, m=D))

            z8 = ar.tile([P, KC, LT], f8, tag="Z8")
            for j in range(KC):
                u = r16()
                nc.vector.scalar_tensor_tensor(u[:, 0:LT], x2[:, j],
                                               vecs[:, 3, j:j + 1],
                                               arep2[:, 0:LT],
                                               AluOpType.mult, AluOpType.mult)
                v = r16()
                nc.vector.scalar_tensor_tensor(v[:, 0:LT], brep2[:, 0:LT],
                                               negs[:, 1, j:j + 1],
                                               u[:, 0:LT],
                                               AluOpType.mult, AluOpType.add)
                nc.vector.tensor_scalar_add(z8[:, j], v[:, 0:LT],
                                            vecs[:, 4, j:j + 1])

            # ---- fc1 + gelu -> h8 ----
            f1c = w8()
            nc.sync.dma_start(f1c[:],
                              f1w8d[:, 2 * KP * 2 * D:3 * KP * 2 * D]
                              .rearrange("p (k i m) -> p k i m", i=2, m=D))
            f1d = w8()
            nc.sync.dma_start(f1d[:],
                              f1w8d[:, 3 * KP * 2 * D:4 * KP * 2 * D]
                              .rearrange("p (k i m) -> p k i m", i=2, m=D))

            h8 = ar.tile([P, 32, LT], f8, tag="H8")

            def fc1_block(wt, mg0):
                for mt in range(KC):
                    mg = mg0 + mt
                    ph = pp()
                    for kp in range(KP):
                        nc.tensor.matmul(ph[:],
                                         wt[:, kp, :, mt * P:(mt + 1) * P],
                                         z8[:, 2 * kp:2 * kp + 2, :],
                                         start=(kp == 0), stop=(kp == KP - 1),
                                         perf_mode=PM.DoubleRow)
                    nc.scalar.activation(h8[:, mg], ph[:], AF.Gelu,
                                         bias=f1bt[:, mg:mg + 1])

            fc1_block(f1a, 0)
            f2a = w8()
            nc.sync.dma_start(f2a[:],
                              f2w8d[:, 0:KP * 2 * D]
                              .rearrange("p (k i m) -> p k i m", i=2, m=D))
            fc1_block(f1b, 8)
            f2b = w8()
            nc.sync.dma_start(f2b[:],
                              f2w8d[:, KP * 2 * D:2 * KP * 2 * D]
                              .rearrange("p (k i m) -> p k i m", i=2, m=D))
            fc1_block(f1c, 16)
            f2c = w8()
            nc.sync.dma_start(f2c[:],
                              f2w8d[:, 2 * KP * 2 * D:3 * KP * 2 * D]
                              .rearrange("p (k i m) -> p k i m", i=2, m=D))
            fc1_block(f1d, 24)
            f2d = w8()
            nc.sync.dma_start(f2d[:],
                              f2w8d[:, 3 * KP * 2 * D:4 * KP * 2 * D]
                              .rearrange("p (k i m) -> p k i m", i=2, m=D))

            # ---- fc2 + gated residual + store ----
            f2t = (f2a, f2b, f2c, f2d)
            for mt in range(KC):
                pz = pp()
                for kp in range(16):
                    nc.tensor.matmul(pz[:],
                                     f2t[kp // 4][:, kp % 4, :,
                                                  mt * P:(mt + 1) * P],
                                     h8[:, 2 * kp:2 * kp + 2, :],
                                     start=(kp == 0), stop=False,
                                     perf_mode=PM.DoubleRow)
                nc.tensor.matmul(pz[:], f2bt[:, mt * P:(mt + 1) * P],
                                 onesrow[:], start=False, stop=True)
                ot = rot.tile([P, LT], f16, tag="OT", bufs=2, name="ot")
                nc.vector.scalar_tensor_tensor(ot[:], pz[:],
                                               vecs[:, 5, mt:mt + 1],
                                               x2[:, mt, :],
                                               AluOpType.mult, AluOpType.add)
                nc.sync.dma_start(outt[mt * P:(mt + 1) * P, :], ot[:])

    _legalize_waits(nc)
    return nc


_NC_CACHE = {}


def _get_nc():
    if "nc" not in _NC_CACHE:
        _NC_CACHE["nc"] = _build()
    return _NC_CACHE["nc"]


F8 = ml_dtypes.float8_e4m3fn


def _feat(v, cols):
    """[D*]-vector -> feature-major [128, cols] (col j = chunk j)."""
    return np.ascontiguousarray(np.asarray(v, np.float32).reshape(cols, P).T)


def _pair8(w):
    """W [Din, M] -> [128, (Din/256)*2*M] fp8: [p][kp][i][m] = W[(2kp+i)*128+p, m]."""
    w = np.asarray(w, np.float32)
    din, m = w.shape
    return np.ascontiguousarray(
        w.reshape(din // 256, 2, P, m).transpose(2, 0, 1, 3)
    ).reshape(P, -1).astype(F8)


def make_in_maps(x, cond, g1_w, g1_b, b1_w, b1_b, a1_w, a1_b,
                 g2_w, g2_b, b2_w, b2_b, a2_w, a2_b,
                 ln1_g, ln1_b, ln2_g, ln2_b,
                 qkv_w, qkv_b, proj_w, proj_b,
                 fc1_w, fc1_b, fc2_w, fc2_b):
    f32, f16 = np.float32, np.float16
    x = np.asarray(x, f32)
    cond = np.asarray(cond, f32)
    qkv_w = np.asarray(qkv_w, f32)
    qkv_b = np.asarray(qkv_b, f32)
    proj_w = np.asarray(proj_w, f32)
    proj_b = np.asarray(proj_b, f32)
    shared = {
        "qw8d": _pair8(qkv_w[:, 0:D]),
        "kw8d": _pair8(qkv_w[:, D:2 * D]),
        "vw8d": _pair8(qkv_w[:, 2 * D:3 * D]),
        # proj lhsT with 64-row k-tiles: [p][c][i][m] = W[(2c+i)*64+p, m]
        "pw8d": np.ascontiguousarray(
            proj_w.reshape(KC, 2, DH, D).transpose(2, 0, 1, 3)
        ).reshape(DH, -1).astype(F8),
        # out-column quarters contiguous so each device tile is [P, KP, 2, D]
        "f1w8d": np.concatenate(
            [_pair8(np.asarray(fc1_w, f32)[:, q * D:(q + 1) * D])
             for q in range(4)], axis=1),
        "f2w8d": _pair8(np.asarray(fc2_w, f32)),
        "modw8d": np.concatenate(
            [_pair8(np.asarray(w, f32)) for w in
             (g1_w, b1_w, a1_w, g2_w, b2_w, a2_w)], axis=1),
        "qbf": _feat(qkv_b[0:D], KC),
        "f1bf": _feat(np.asarray(fc1_b, f32), 32),
        # v bias folds into the proj bias: softmax rows sum to 1
        "pbrow": (proj_b + qkv_b[2 * D:3 * D] @ proj_w)[None, :].astype(f16),
        "f2brow": np.asarray(fc2_b, f16)[None, :],
        "modbf": np.hstack([_feat(v, KC) for v in
                            (g1_b, b1_b, a1_b, g2_b, b2_b, a2_b)]),
        "lnf": np.hstack([_feat(v, KC) for v in
                          (ln1_g, ln1_b, ln2_g, ln2_b)]),
    }
    in_maps = []
    for c in range(8):
        b, h = c // 2, c % 2
        xb = x[b].T  # [D, NT]
        perm = np.concatenate([np.arange(h * LT, (h + 1) * LT),
                               np.arange((1 - h) * LT, (2 - h) * LT)])
        m = dict(shared)
        m["xt16"] = np.ascontiguousarray(xb[:, perm]).astype(f16)
        m["cond8d"] = _feat(cond[b], KC).astype(F8)
        in_maps.append(m)
    return in_maps


def kernel(**inputs):
    nc = _get_nc()
    in_maps = make_in_maps(**inputs)
    res = run_bass_kernel_spmd(nc, in_maps, list(range(8)))
    out = np.empty((B, NT, D), np.float32)
    for c in range(8):
        b, h = c // 2, c % 2
        out[b, h * LT:(h + 1) * LT, :] = res.results[c]["outt"].T.astype(
            np.float32)
    return out
